# revision 9
# baseline (speedup 1.0000x reference)
"""Trainium2 kernel for nn_Decoder_featurizer: HRR decoder featurization.

reference: out = concat([p, l, assoc(dec_d, p)..., assoc(dec_d, l)...], -1)
where assoc(d, x)[j] = sum_t d[t] * x[(j+t) % N]  (circular correlation).

Circular correlation is a circulant matmul: Y[b, d*N+j] = sum_k X[b,k] * W[k, d*N+j]
with W[k, d*N+j] = dec[d, (k-j) % N].  Run as a dense bf16 matmul on the tensor
engine, data-parallel over batch across 8 NeuronCores.

Key structure: each decoder's circulant W_d is tile-circulant — its 8x8 grid of
128x128 tiles has only 8 distinct tiles T_d^(r), r=(ktile - jtile) mod 8.  So
all weights live SBUF-resident in an extended per-decoder buffer (11 tile slots
holding T^(7),T^(6),...,T^(0),T^(7),T^(6),T^(5)) and any (ktile, 4-consecutive
jtiles) run of W is a contiguous 512-wide slice.  No weight streaming.

Pipeline (v2): 2048-col output chunks, one 4-bank PSUM tile per chunk,
double-buffered, so the DVE drain of chunk c overlaps the matmuls of chunk
c+1.  Redundant per-matmul LDWEIGHTS are deduped at the BIR level
(_dedup_ldweights) since walrus's --enable-ldw-opt rejects pre-split pairs.
"""

import numpy as np
import ml_dtypes

HRR = 1024
D = 16
B = 8192
NCORES = 8
BPC = B // NCORES            # batch rows per core
ROWS = 2 * BPC               # matmul rows per core (problem + lemma stacked)
DN = D * HRR                 # 16384 assoc features per input
OUT_COLS = 2 * HRR + 2 * DN  # 34816
WSLOTS = 11                  # extended circulant buffer: 8 + 3 wrap slots
WEXT = WSLOTS * 128          # 1408

_CACHE = {}


def _build_program(loop_iters: int = 1):
    import contextlib
    import concourse.bacc as bacc
    import concourse.mybir as mybir
    from concourse.tile import TileContext

    nc = bacc.Bacc("TRN2", target_bir_lowering=False, debug=False,
                   num_devices=NCORES)
    xT = nc.dram_tensor("xT", [HRR, ROWS], mybir.dt.bfloat16,
                        kind="ExternalInput").ap()
    wext = nc.dram_tensor("wcirc", [128, D * WEXT], mybir.dt.bfloat16,
                          kind="ExternalInput").ap()
    xf = nc.dram_tensor("xf", [BPC, 2 * HRR], mybir.dt.float32,
                        kind="ExternalInput").ap()
    out = nc.dram_tensor("out", [BPC, OUT_COLS], mybir.dt.float32,
                         kind="ExternalOutput").ap()

    NT = DN // 512   # 32 n-tiles of 512 (2 per decoder)
    KT = HRR // 128  # 8 k-tiles

    with TileContext(nc) as tc:
        with (
            tc.tile_pool(name="xp", bufs=1) as xpool,
            tc.tile_pool(name="wp", bufs=1) as wpool,
            tc.tile_pool(name="ps", bufs=8, space="PSUM") as pspool,
            tc.tile_pool(name="ob", bufs=8) as opool,
            tc.tile_pool(name="pt", bufs=2) as ptpool,
        ):
            # passthrough columns: out[:, :2048] = [problem, lemma] rows (f32)
            for m in range(BPC // 128):
                t = ptpool.tile([128, 2 * HRR], mybir.dt.float32)
                nc.sync.dma_start(out=t[:], in_=xf[m * 128:(m + 1) * 128, :])
                nc.sync.dma_start(out=out[m * 128:(m + 1) * 128, 0:2 * HRR],
                                  in_=t[:])

            # resident transposed activations: 8 tiles [128k, 2048b] bf16
            xtiles = []
            for k in range(KT):
                t = xpool.tile([128, ROWS], mybir.dt.bfloat16, tag=f"x{k}")
                nc.sync.dma_start(out=t[:], in_=xT[k * 128:(k + 1) * 128, :])
                xtiles.append(t)

            # resident circulant weight tiles: 16 decoders x [128, 1408] bf16
            wtiles = []
            for d in range(D):
                t = wpool.tile([128, WEXT], mybir.dt.bfloat16, tag=f"w{d}")
                nc.sync.dma_start(out=t[:],
                                  in_=wext[:, d * WEXT:(d + 1) * WEXT])
                wtiles.append(t)

            # dense matmul: [2048, 1024] @ [1024, 16384], weights resident.
            # 2 psum tiles x [128, 2048] (4 banks each): one LDWEIGHTS per
            # 4 matmuls; 512KB output DMAs.
            # (loop_iters > 1 repeats the compute on-device, for timing)
            loop_cm = (tc.For_i(0, loop_iters, 1,
                                hint_engines=(mybir.EngineType.PE,
                                              mybir.EngineType.SP,
                                              mybir.EngineType.DVE))
                       if loop_iters > 1 else contextlib.nullcontext())
            with loop_cm:
              for m in range(16):       # one m-tile per pass
                for qq in range(DN // 4096):  # 4 chunks of 4096 out cols
                    # 2 psum tiles cover 4096 cols: one LDWEIGHTS feeds
                    # 8 consecutive matmuls
                    psums = [pspool.tile([128, 2048], mybir.dt.float32,
                                         name=f"ps{pi}", tag=f"ps{pi}",
                                         bufs=1)
                             for pi in range(2)]
                    for k in range(KT):
                        lhsT = xtiles[k][:, m * 128:(m + 1) * 128]
                        for jj in range(8):
                            n512 = 8 * qq + jj
                            d, h = n512 // 2, n512 % 2
                            s0 = 7 - ((k - 4 * h) % 8)
                            nc.tensor.matmul(
                                psums[jj // 4][:, (jj % 4) * 512:
                                               (jj % 4 + 1) * 512],
                                lhsT,
                                wtiles[d][:, s0 * 128:s0 * 128 + 512],
                                start=(k == 0), stop=(k == KT - 1))
                    col0 = 2 * HRR + (DN if m >= 8 else 0) + qq * 4096
                    row0 = (m % 8) * 128
                    for pi in range(2):
                        for half in range(2):
                            ot = opool.tile([128, 1024], mybir.dt.float32)
                            nc.vector.tensor_copy(
                                out=ot[:],
                                in_=psums[pi][:, half * 1024:
                                              (half + 1) * 1024])
                            nc.sync.dma_start(
                                out=out[row0:row0 + 128,
                                        col0 + (2 * pi + half) * 1024:
                                        col0 + (2 * pi + half + 1) * 1024],
                                in_=ot[:])
    _finalize_with_dedup(nc)
    return nc


def _dedup_ldweights(nc):
    """Drop redundant InstLdweights from the PE stream.

    bacc emits every matmul as an (InstLdweights, InstMatmult) pair; the
    matmult is non-self-loading, so the PE weight register persists across
    matmuls.  Consecutive pairs with an identical stationary AP reload the
    same weights (~107ns each on HW).  walrus's --enable-ldw-opt dedup
    rejects pre-split InstLdweights, so dedup here instead: drop an
    InstLdweights when its signature matches the previous one on the PE
    stream AND it carries no waits/updates (a rewritten stationary buffer
    would carry a wait; ours are loaded once and immutable anyway).
    Conservatively resets tracking at block boundaries and on any other PE
    instruction.
    """
    import concourse.mybir as mybir

    InstLdweights = mybir.InstLdweights
    InstMatmult = mybir.InstMatmult
    n_drop = 0
    for fn in nc.m.functions:
        for blk in fn.blocks:
            keep = []
            last_sig = None
            for inst in blk.instructions:
                if isinstance(inst, InstLdweights):
                    pap = inst.ins[0]
                    sig = (pap.memref, pap.offset, str(pap.ap),
                           str(pap.dtype),
                           str(getattr(inst, "perf_mode", None)),
                           str(getattr(inst, "is_transpose", None)),
                           str(getattr(inst, "tile_position", None)))
                    si = inst.sync_info
                    bare = si is None or (len(si.on_wait) == 0
                                          and len(si.on_update) == 0)
                    if sig == last_sig and bare:
                        n_drop += 1
                        continue
                    last_sig = sig
                elif getattr(inst, "engine", None) == mybir.EngineType.PE:
                    if isinstance(inst, InstMatmult):
                        if getattr(inst, "is_transpose", None):
                            last_sig = None
                    else:
                        last_sig = None
                keep.append(inst)
            if n_drop:
                try:
                    blk.instructions = keep
                except Exception:
                    insts = blk.instructions
                    while len(insts):
                        insts.pop()
                    for i in keep:
                        insts.append(i)
    return n_drop


def _finalize_with_dedup(nc):
    orig_mv = nc.move_matmul_waits_to_ldweights

    def _mv():
        orig_mv()
        _dedup_ldweights(nc)

    nc.move_matmul_waits_to_ldweights = _mv
    nc.finalize()


def _build_program_v2(loop_iters: int = 1):
    """Ping-pong PSUM variant: 2048-col chunks, one [128,2048] psum tile per
    chunk double-buffered across chunks, so the DVE drain of chunk c overlaps
    the matmuls of chunk c+1 instead of stalling the PE at every 4096-col
    boundary.  Stationary (x) reuse drops from 8 to 4 matmuls per LDWEIGHTS;
    with walrus ldw-dedup enabled that costs ~27us but removes the per-chunk
    PE stall."""
    import contextlib
    import concourse.bacc as bacc
    import concourse.mybir as mybir
    from concourse.tile import TileContext

    nc = bacc.Bacc("TRN2", target_bir_lowering=False, debug=False,
                   num_devices=NCORES)
    xT = nc.dram_tensor("xT", [HRR, ROWS], mybir.dt.bfloat16,
                        kind="ExternalInput").ap()
    wext = nc.dram_tensor("wcirc", [128, D * WEXT], mybir.dt.bfloat16,
                          kind="ExternalInput").ap()
    xf = nc.dram_tensor("xf", [BPC, 2 * HRR], mybir.dt.float32,
                        kind="ExternalInput").ap()
    out = nc.dram_tensor("out", [BPC, OUT_COLS], mybir.dt.float32,
                         kind="ExternalOutput").ap()

    KT = HRR // 128  # 8 k-tiles

    with TileContext(nc) as tc:
        with (
            tc.tile_pool(name="xp", bufs=1) as xpool,
            tc.tile_pool(name="wp", bufs=1) as wpool,
            tc.tile_pool(name="ps", bufs=2, space="PSUM") as pspool,
            tc.tile_pool(name="ob", bufs=4) as opool,
            tc.tile_pool(name="pt", bufs=2) as ptpool,
        ):
            # passthrough columns: out[:, :2048] = [problem, lemma] rows (f32)
            for m in range(BPC // 128):
                t = ptpool.tile([128, 2 * HRR], mybir.dt.float32)
                nc.sync.dma_start(out=t[:], in_=xf[m * 128:(m + 1) * 128, :])
                nc.sync.dma_start(out=out[m * 128:(m + 1) * 128, 0:2 * HRR],
                                  in_=t[:])

            xtiles = []
            for k in range(KT):
                t = xpool.tile([128, ROWS], mybir.dt.bfloat16, tag=f"x{k}")
                nc.sync.dma_start(out=t[:], in_=xT[k * 128:(k + 1) * 128, :])
                xtiles.append(t)

            wtiles = []
            for d in range(D):
                t = wpool.tile([128, WEXT], mybir.dt.bfloat16, tag=f"w{d}")
                nc.sync.dma_start(out=t[:],
                                  in_=wext[:, d * WEXT:(d + 1) * WEXT])
                wtiles.append(t)

            loop_cm = (tc.For_i(0, loop_iters, 1,
                                hint_engines=(mybir.EngineType.PE,
                                              mybir.EngineType.SP,
                                              mybir.EngineType.DVE))
                       if loop_iters > 1 else contextlib.nullcontext())
            with loop_cm:
              for m in range(16):
                for cc in range(8):          # 8 chunks of 2048 out cols
                    ps = pspool.tile([128, 2048], mybir.dt.float32,
                                     name="ps")
                    for k in range(KT):
                        lhsT = xtiles[k][:, m * 128:(m + 1) * 128]
                        for jj in range(4):
                            n512 = 4 * cc + jj
                            d, h = n512 // 2, n512 % 2
                            s0 = 7 - ((k - 4 * h) % 8)
                            nc.tensor.matmul(
                                ps[:, jj * 512:(jj + 1) * 512],
                                lhsT,
                                wtiles[d][:, s0 * 128:s0 * 128 + 512],
                                start=(k == 0), stop=(k == KT - 1))
                    col0 = 2 * HRR + (DN if m >= 8 else 0) + cc * 2048
                    row0 = (m % 8) * 128
                    ot = opool.tile([128, 2048], mybir.dt.float32, name="ot")
                    nc.vector.tensor_copy(out=ot[:], in_=ps[:])
                    nc.sync.dma_start(
                        out=out[row0:row0 + 128, col0:col0 + 2048],
                        in_=ot[:])
    _finalize_with_dedup(nc)
    return nc


def _get_program(loop_iters: int = 1):
    key = f"nc{loop_iters}"
    if key not in _CACHE:
        _CACHE[key] = _build_program_v2(loop_iters)
    return _CACHE[key]


def _build_weights(decoders: np.ndarray) -> np.ndarray:
    """Extended circulant tile buffer [128, D*1408] bf16.

    Slot s of decoder d holds T_d^{(7-s) mod 8} where
    T_d^{(r)}[p, q] = dec[d, (r*128 + p - q) % 1024].
    """
    p = np.arange(128)[:, None]
    q = np.arange(128)[None, :]
    out = np.empty((128, D * WEXT), dtype=ml_dtypes.bfloat16)
    for d in range(D):
        for s in range(WSLOTS):
            r = (7 - s) % 8
            tile = decoders[d][(r * 128 + p - q) % HRR]
            out[:, d * WEXT + s * 128: d * WEXT + (s + 1) * 128] = tile
    return out


def kernel(problemhrr: np.ndarray, lemmahrr: np.ndarray,
           decoders: np.ndarray) -> np.ndarray:
    from concourse.bass_utils import run_bass_kernel_spmd

    problemhrr = np.asarray(problemhrr, dtype=np.float32)
    lemmahrr = np.asarray(lemmahrr, dtype=np.float32)
    decoders = np.asarray(decoders, dtype=np.float32)

    nc = _get_program()
    w = _build_weights(decoders)

    in_maps = []
    for c in range(NCORES):
        p = problemhrr[c * BPC:(c + 1) * BPC]   # [1024, 1024]
        l = lemmahrr[c * BPC:(c + 1) * BPC]
        x = np.concatenate([p, l], axis=0)       # [2048, 1024]
        xT = np.ascontiguousarray(x.T).astype(ml_dtypes.bfloat16)
        xf = np.concatenate([p, l], axis=1)      # [1024, 2048] f32
        in_maps.append({"xT": xT, "wcirc": w, "xf": np.ascontiguousarray(xf)})

    res = run_bass_kernel_spmd(nc, in_maps, list(range(NCORES)))
    return np.concatenate([res.results[c]["out"] for c in range(NCORES)],
                          axis=0)

